# revision 38
# baseline (speedup 1.0000x reference)
"""Trainium2 Bass kernel for nn_ComplexFaberConv (gnn_message_passing).

Strategy
--------
The K-hop einsum collapses on the host (sum_k s_k W[k] -> one effective
128x128 weight per real/imag part), and the degree normalization factorizes
per edge as val_e = a[dst] * b[src].  The whole op then reduces to

    out[n, :] = afull[n] * sum_{fwd e: dst=n} G_f[src(e)]
              + bfull[n] * sum_{bwd e: dst=n} G_b[src(e)]          (+ bias)

where G_f/G_b are per-node 256-wide feature rows (x already pushed through
the effective weights and scaled by the src-side degree factor).

Device layout (8 cores, SPMD):
  phase 1  - each core receives only its shard of x (fp16, transposed) and
             computes its [2*nsh, 256] slice of the gather table
             (G_f rows then G_b rows) with two fp16 matmuls per 128-row tile.
  AllGather- the 8 table shards are gathered device-side into the full
             [16*nsh, 256] table on every core (fp16, ~100 MB).
  phase 2  - per 128-node dst tile, gather the tile's edges in 128-edge
             chunks (indirect DMA from the gathered table), build a
             selection matrix sel[e, d] = (dst_slot[e] == d) via one DVE
             is_equal against an on-device iota, and accumulate
             psum[128, 256] += sel.T @ gathered on the tensor engine.
             Two PSUM accumulators (fwd/bwd) get the per-node a/b scales
             and are summed and written out in fp16.

Host work is only indexing (degree counts, a load-balancing snake
permutation of nodes into 128-slot bins, per-bin edge chunking) plus int8
quantization.  x ships as int8 with a shared per-node scale (~26 MB total
instead of 8 replicated 205 MB f32 tables); the dequant factor folds into
the phase-1 activation scales.  The output returns as int8 with a per-node
scale (~26 MB; absmax-rel error bounded by ~1/254 data-independently).
Host un-permutes, dequantizes, and adds the (exact, f32) bias row.
"""
import numpy as np

import concourse.bass as bass
import concourse.bacc as bacc
import concourse.mybir as mybir
import concourse.tile as tile
from concourse import bass_utils

K = 3
ALPHA = 0.5
EXPONENT = -0.25
NCORES = 8
P = 128
DCAT = 256  # real||imag feature width

# set by tests to run the multi-core simulator instead of hardware
_SIM = False

_prog_cache = {}
_last_info = {}


# --------------------------------------------------------------------------
# host-side preparation (indexing only -- no dense math)
# --------------------------------------------------------------------------

def _host_prep(x_real, x_imag, W_real, W_imag, b_real, b_imag, edge_index):
    n = x_real.shape[0]
    row = edge_index[0].astype(np.int64)
    col = edge_index[1].astype(np.int64)
    tpc = -(-n // (NCORES * P))  # 128-row tiles per core
    nbins = NCORES * tpc
    nsh = tpc * P                # padded nodes per core

    deg_out = np.bincount(row, minlength=n).astype(np.float32)
    deg_in = np.bincount(col, minlength=n).astype(np.float32)
    with np.errstate(divide="ignore"):
        afull = np.where(deg_out > 0, deg_out ** np.float32(EXPONENT), 0.0)
        bfull = np.where(deg_in > 0, deg_in ** np.float32(EXPONENT), 0.0)
    afull = afull.astype(np.float32)
    bfull = bfull.astype(np.float32)

    s = (0.5 ** np.arange(K)).astype(np.float32)
    WrT = np.einsum("kod,k->do", W_real, s).astype(np.float32)
    WiT = np.einsum("kod,k->do", W_imag, s).astype(np.float32)
    Z = np.zeros_like(WrT)
    # table row = [G_f_real | G_f_imag | G_b_real | G_b_imag]
    rhs_r = np.concatenate([0.5 * WrT, WiT, 0.5 * WrT, Z], axis=1).astype(np.float16)
    rhs_i = np.concatenate([-0.5 * WiT, 0.5 * WrT, -0.5 * WiT, 0.5 * WrT],
                           axis=1).astype(np.float16)
    c1 = (s @ b_real - s @ b_imag).astype(np.float32)
    c2 = (s @ b_real + s @ b_imag).astype(np.float32)

    # ---- balance nodes into (core, tile) bins of 128 slots: snake over
    # load-sorted nodes gives near-equal per-bin edge counts, vectorized.
    load = deg_out + deg_in
    order = np.argsort(-load, kind="stable")
    node_bin = np.empty(n, np.int64)
    node_slot = np.empty(n, np.int64)
    nfull = n // nbins
    idx = np.arange(nfull * nbins)
    r = idx // nbins
    b = idx % nbins
    node_bin[order[idx]] = np.where(r % 2 == 0, b, nbins - 1 - b)
    node_slot[order[idx]] = r
    rem = n - nfull * nbins
    if rem:
        i2 = np.arange(rem)
        node_bin[order[nfull * nbins + i2]] = (
            i2 if nfull % 2 == 0 else nbins - 1 - i2)
        node_slot[order[nfull * nbins + i2]] = nfull
    gslot = node_bin * P + node_slot
    core_of = node_bin // tpc
    lrow = (node_bin % tpc) * P + node_slot

    # x shards, feature-major (matmul lhsT), int8 with a per-node scale
    # shared between real/imag so dequant folds into the phase-1 row scale
    s_row = np.maximum(np.abs(x_real).max(axis=1), np.abs(x_imag).max(axis=1))
    s_row = np.maximum(s_row / np.float32(127.0), np.float32(1e-12))
    xr_q = np.clip(np.rint(x_real / s_row[:, None]), -127, 127).astype(np.int8)
    xi_q = np.clip(np.rint(x_imag / s_row[:, None]), -127, 127).astype(np.int8)
    xr_sh = np.zeros((NCORES, nsh, P), np.int8)
    xi_sh = np.zeros((NCORES, nsh, P), np.int8)
    xr_sh[core_of, lrow] = xr_q
    xi_sh[core_of, lrow] = xi_q
    xrT = np.ascontiguousarray(xr_sh.transpose(0, 2, 1))
    xiT = np.ascontiguousarray(xi_sh.transpose(0, 2, 1))

    # ---- edge chunking: per dst bin, 128-edge chunks (fwd then bwd)
    fwd_cnt = np.bincount(node_bin[row], minlength=nbins)
    bwd_cnt = np.bincount(node_bin[col], minlength=nbins)
    cf = int(-(-fwd_cnt.max() // P))
    cb = int(-(-bwd_cnt.max() // P))
    cpt = cf + cb
    nch = tpc * cpt

    # table rows after AllGather: core blocks of [G_f (nsh) | G_b (nsh)]
    gtab_f = core_of * (2 * nsh) + (node_bin % tpc) * P + node_slot
    gtab_b = gtab_f + nsh

    # srcs packs (dst_slot << 18) | table_row; padding slot=255 never matches
    srcs = np.full((NCORES, P, nch), 255 << 18, np.int32)
    for direction in range(2):
        dst = row if direction == 0 else col
        src = col if direction == 0 else row
        tabrow = (gtab_f if direction == 0 else gtab_b)[src]
        dbin = node_bin[dst]
        eorder = np.argsort(dbin, kind="stable")
        dbin_s = dbin[eorder]
        slot_s = node_slot[dst][eorder]
        tab_s = tabrow[eorder]
        starts = np.searchsorted(dbin_s, np.arange(nbins + 1))
        rr = np.arange(dst.shape[0]) - starts[dbin_s]
        cbase = 0 if direction == 0 else cf
        colidx = (dbin_s % tpc) * cpt + cbase + rr // P
        corei = dbin_s // tpc
        srcs[corei, rr % P, colidx] = (slot_s.astype(np.int32) << 18) | tab_s

    afac = np.zeros((NCORES, P, tpc), np.float16)
    bfac = np.zeros((NCORES, P, tpc), np.float16)
    afac[core_of, node_slot, node_bin % tpc] = afull.astype(np.float16)
    bfac[core_of, node_slot, node_bin % tpc] = bfull.astype(np.float16)
    # phase-1 scales with the x dequant factor folded in
    afac1 = np.zeros((NCORES, P, tpc), np.float16)
    bfac1 = np.zeros((NCORES, P, tpc), np.float16)
    afac1[core_of, node_slot, node_bin % tpc] = (afull * s_row).astype(np.float16)
    bfac1[core_of, node_slot, node_bin % tpc] = (bfull * s_row).astype(np.float16)

    return dict(xrT=xrT, xiT=xiT, rhs_r=rhs_r, rhs_i=rhs_i, srcs=srcs,
                afac=afac, bfac=bfac, afac1=afac1, bfac1=bfac1,
                c1=c1, c2=c2, gslot=gslot,
                cf=cf, cb=cb, tpc=tpc, n=n, nsh=nsh)


# --------------------------------------------------------------------------
# device program
# --------------------------------------------------------------------------

def _build_program(tpc, cf, cb, rhs_r, rhs_i):
    nsh = tpc * P
    ntot = NCORES * nsh
    cpt = cf + cb
    nch = tpc * cpt
    nc = bacc.Bacc("TRN2", target_bir_lowering=False, debug=False)
    f16 = mybir.dt.float16
    f32 = mybir.dt.float32
    xrT = nc.dram_tensor("xrT", [P, nsh], mybir.dt.int8, kind="ExternalInput").ap()
    xiT = nc.dram_tensor("xiT", [P, nsh], mybir.dt.int8, kind="ExternalInput").ap()
    # the tiny folded weights are identical on every core: embed them in the
    # NEFF (Const, loaded once at model-load) instead of uploading per run
    rhsr = nc.inline_tensor(np.ascontiguousarray(rhs_r), "rhsr_c").ap()
    rhsi = nc.inline_tensor(np.ascontiguousarray(rhs_i), "rhsi_c").ap()
    srcs = nc.dram_tensor("srcs", [P, nch], mybir.dt.int32, kind="ExternalInput").ap()
    afac = nc.dram_tensor("afac", [P, tpc], f16, kind="ExternalInput").ap()
    bfac = nc.dram_tensor("bfac", [P, tpc], f16, kind="ExternalInput").ap()
    afac1 = nc.dram_tensor("afac1", [P, tpc], f16, kind="ExternalInput").ap()
    bfac1 = nc.dram_tensor("bfac1", [P, tpc], f16, kind="ExternalInput").ap()
    # int8 output with a per-node scale halves the output transfer; the
    # absmax-rel error this adds is bounded by 1/254 regardless of data
    outq = nc.dram_tensor("outq", [nsh, DCAT], mybir.dt.int8,
                          kind="ExternalOutput").ap()
    outs = nc.dram_tensor("outs", [nsh, 1], f32, kind="ExternalOutput").ap()

    with tile.TileContext(nc) as tc:
        with (
            tc.tile_pool(name="meta", bufs=1) as meta_tp,
            tc.tile_pool(name="dram", bufs=1, space="DRAM") as dram_tp,
            tc.tile_pool(name="g", bufs=8) as g_tp,
            tc.tile_pool(name="sel", bufs=8) as sel_tp,
            tc.tile_pool(name="tb", bufs=3) as tb_tp,
            tc.tile_pool(name="post", bufs=3) as post_tp,
            tc.tile_pool(name="ps1", bufs=2, space="PSUM") as ps1_tp,
            tc.tile_pool(name="ps", bufs=2, space="PSUM") as ps_tp,
        ):
            xq_sb = meta_tp.tile([P, 2 * nsh], mybir.dt.int8)
            nc.sync.dma_start(out=xq_sb[:, 0:nsh], in_=xrT[:])
            nc.sync.dma_start(out=xq_sb[:, nsh:2 * nsh], in_=xiT[:])
            xrT_sb = meta_tp.tile([P, nsh], f16)
            nc.scalar.copy(out=xrT_sb[:], in_=xq_sb[:, 0:nsh])
            xiT_sb = meta_tp.tile([P, nsh], f16)
            nc.scalar.copy(out=xiT_sb[:], in_=xq_sb[:, nsh:2 * nsh])
            rhsr_sb = meta_tp.tile([P, 4 * P], f16)
            nc.sync.dma_start(out=rhsr_sb[:], in_=rhsr[:])
            rhsi_sb = meta_tp.tile([P, 4 * P], f16)
            nc.sync.dma_start(out=rhsi_sb[:], in_=rhsi[:])
            srcsp_sb = meta_tp.tile([P, nch], mybir.dt.int32)
            nc.sync.dma_start(out=srcsp_sb[:], in_=srcs[:])
            afac_h = meta_tp.tile([P, tpc], f16)
            nc.sync.dma_start(out=afac_h[:], in_=afac[:])
            bfac_h = meta_tp.tile([P, tpc], f16)
            nc.sync.dma_start(out=bfac_h[:], in_=bfac[:])
            afac_sb = meta_tp.tile([P, tpc], f32)
            nc.scalar.copy(out=afac_sb[:], in_=afac_h[:])
            bfac_sb = meta_tp.tile([P, tpc], f32)
            nc.scalar.copy(out=bfac_sb[:], in_=bfac_h[:])
            afac1_h = meta_tp.tile([P, tpc], f16)
            nc.sync.dma_start(out=afac1_h[:], in_=afac1[:])
            bfac1_h = meta_tp.tile([P, tpc], f16)
            nc.sync.dma_start(out=bfac1_h[:], in_=bfac1[:])
            afac1_sb = meta_tp.tile([P, tpc], f32)
            nc.scalar.copy(out=afac1_sb[:], in_=afac1_h[:])
            bfac1_sb = meta_tp.tile([P, tpc], f32)
            nc.scalar.copy(out=bfac1_sb[:], in_=bfac1_h[:])
            iota_sb = meta_tp.tile([P, P], mybir.dt.int32)
            nc.gpsimd.iota(iota_sb[:], [[1, P]], channel_multiplier=0)
            # unpack (slot << 18) | row
            rows_sb = meta_tp.tile([P, nch], mybir.dt.int32)
            nc.vector.tensor_scalar(
                out=rows_sb[:], in0=srcsp_sb[:], scalar1=0x3FFFF, scalar2=None,
                op0=mybir.AluOpType.bitwise_and)
            slots_sb = meta_tp.tile([P, nch], mybir.dt.int32)
            nc.vector.tensor_scalar(
                out=slots_sb[:], in0=srcsp_sb[:], scalar1=18, scalar2=None,
                op0=mybir.AluOpType.logical_shift_right)

            tabshard = dram_tp.tile([2 * nsh, DCAT], f16)
            tabfull = dram_tp.tile([2 * ntot, DCAT], f16, addr_space="Shared")

            # ---- phase 1: build local table shard
            for t in range(tpc):
                sl = slice(t * P, (t + 1) * P)
                ps = ps1_tp.tile([P, 4 * P], f32, space="PSUM", tag="p1")
                nc.tensor.matmul(out=ps[:], lhsT=xrT_sb[:, sl], rhs=rhsr_sb[:],
                                 start=True, stop=False)
                nc.tensor.matmul(out=ps[:], lhsT=xiT_sb[:, sl], rhs=rhsi_sb[:],
                                 start=False, stop=True)
                tb = tb_tp.tile([P, 4 * P], f16, tag="tb")
                nc.scalar.activation(
                    out=tb[:, 0:2 * P], in_=ps[:, 0:2 * P],
                    func=mybir.ActivationFunctionType.Copy,
                    scale=bfac1_sb[:, t:t + 1])
                nc.scalar.activation(
                    out=tb[:, 2 * P:4 * P], in_=ps[:, 2 * P:4 * P],
                    func=mybir.ActivationFunctionType.Copy,
                    scale=afac1_sb[:, t:t + 1])
                nc.sync.dma_start(out=tabshard[sl], in_=tb[:, 0:DCAT])
                nc.sync.dma_start(out=tabshard[nsh + t * P:nsh + (t + 1) * P],
                                  in_=tb[:, DCAT:2 * DCAT])

            # ---- all-gather table shards across the 8 cores
            nc.gpsimd.collective_compute(
                "AllGather", mybir.AluOpType.bypass,
                replica_groups=[list(range(NCORES))],
                ins=[tabshard.opt()], outs=[tabfull.opt()])

            # ---- phase 2: gather + segment-sum via sel-matmul
            # NOTE: one indirect gather per 128-edge chunk.  The hardware
            # reads exactly ONE offset per partition per instruction (a
            # multi-column offset AP silently degrades to offset[:, 0] plus a
            # contiguous run), so chunks cannot be batched into one
            # instruction.
            for t in range(tpc):
                pf = ps_tp.tile([P, DCAT], f32, space="PSUM", tag="pf")
                pb = ps_tp.tile([P, DCAT], f32, space="PSUM", tag="pb")
                for c in range(cpt):
                    colx = t * cpt + c
                    gt = g_tp.tile([P, DCAT], f16, tag="gt")
                    nc.gpsimd.indirect_dma_start(
                        out=gt[:], out_offset=None, in_=tabfull[:],
                        in_offset=bass.IndirectOffsetOnAxis(
                            ap=rows_sb[:, colx:colx + 1], axis=0))
                    sel = sel_tp.tile([P, P], f16, tag="sel")
                    nc.vector.tensor_tensor(
                        out=sel[:],
                        in0=slots_sb[:, colx:colx + 1].to_broadcast([P, P]),
                        in1=iota_sb[:],
                        op=mybir.AluOpType.is_equal)
                    tgt = pf if c < cf else pb
                    nc.tensor.matmul(
                        out=tgt[:], lhsT=sel[:], rhs=gt[:],
                        start=(c == 0 or c == cf),
                        stop=(c == cf - 1 or c == cpt - 1))
                s1 = post_tp.tile([P, DCAT], f32, tag="s1")
                nc.scalar.activation(
                    out=s1[:], in_=pf[:],
                    func=mybir.ActivationFunctionType.Copy,
                    scale=afac_sb[:, t:t + 1])
                s2 = post_tp.tile([P, DCAT], f32, tag="s2")
                nc.vector.tensor_scalar_mul(
                    out=s2[:], in0=pb[:], scalar1=bfac_sb[:, t:t + 1])
                ot = post_tp.tile([P, DCAT], f32, tag="ot")
                nc.vector.tensor_tensor(
                    out=ot[:], in0=s1[:], in1=s2[:], op=mybir.AluOpType.add)
                # per-node int8 quantization: q = ot * 127/rowabsmax
                ra = post_tp.tile([P, 1], f32, tag="ra")
                nc.vector.tensor_reduce(
                    out=ra[:], in_=ot[:], axis=mybir.AxisListType.X,
                    op=mybir.AluOpType.max, apply_absolute_value=True)
                rc = post_tp.tile([P, 1], f32, tag="rc")
                nc.vector.tensor_scalar_max(out=rc[:], in0=ra[:], scalar1=1e-12)
                ri = post_tp.tile([P, 1], f32, tag="ri")
                nc.vector.reciprocal(out=ri[:], in_=rc[:])
                nc.vector.tensor_scalar_mul(out=ri[:], in0=ri[:], scalar1=127.0)
                # the hardware f32->int8 convert rounds to nearest; no
                # explicit pre-rounding (CoreSim truncates and reads ~2x the
                # HW quantization error -- hardware is truth here)
                q = post_tp.tile([P, DCAT], mybir.dt.int8, tag="q")
                nc.vector.tensor_scalar_mul(out=q[:], in0=ot[:], scalar1=ri[:])
                nc.sync.dma_start(out=outq[t * P:(t + 1) * P], in_=q[:])
                nc.sync.dma_start(out=outs[t * P:(t + 1) * P], in_=ri[:])
    nc.compile()
    return nc


def _get_program(tpc, cf, cb, rhs_r, rhs_i):
    # weights are baked into the NEFF as consts, so they are part of the key
    key = (tpc, cf, cb, rhs_r.tobytes(), rhs_i.tobytes())
    if key not in _prog_cache:
        _prog_cache[key] = _build_program(tpc, cf, cb, rhs_r, rhs_i)
    return _prog_cache[key]


# --------------------------------------------------------------------------
# entry point
# --------------------------------------------------------------------------

def kernel(x_real, x_imag, W_real, W_imag, b_real, b_imag, edge_index):
    import time
    t0 = time.time()
    x_real = np.asarray(x_real, dtype=np.float32)
    x_imag = np.asarray(x_imag, dtype=np.float32)
    W_real = np.asarray(W_real, dtype=np.float32)
    W_imag = np.asarray(W_imag, dtype=np.float32)
    b_real = np.asarray(b_real, dtype=np.float32)
    b_imag = np.asarray(b_imag, dtype=np.float32)
    edge_index = np.asarray(edge_index)

    prep = _host_prep(x_real, x_imag, W_real, W_imag, b_real, b_imag, edge_index)
    t1 = time.time()
    nc = _get_program(prep["tpc"], prep["cf"], prep["cb"],
                      prep["rhs_r"], prep["rhs_i"])
    t2 = time.time()

    in_maps = []
    for ci in range(NCORES):
        in_maps.append({
            "xrT": prep["xrT"][ci],
            "xiT": prep["xiT"][ci],
            "srcs": prep["srcs"][ci],
            "afac": prep["afac"][ci],
            "bfac": prep["bfac"][ci],
            "afac1": prep["afac1"][ci],
            "bfac1": prep["bfac1"][ci],
        })

    if _SIM:
        from concourse import bass_interp
        sim = bass_interp.MultiCoreSim(nc, num_cores=NCORES)
        for ci in range(NCORES):
            core = sim.cores[ci]
            for k, v in in_maps[ci].items():
                core.tensor(k)[:] = v
        sim.simulate()
        outqs = [np.array(sim.cores[ci].tensor("outq")) for ci in range(NCORES)]
        outss = [np.array(sim.cores[ci].tensor("outs")) for ci in range(NCORES)]
    else:
        res = bass_utils.run_bass_kernel_spmd(
            nc, in_maps, core_ids=list(range(NCORES)))
        outqs = [r["outq"] for r in res.results]
        outss = [r["outs"] for r in res.results]
    t3 = time.time()
    _last_info["prep_s"] = t1 - t0
    _last_info["compile_s"] = t2 - t1
    _last_info["exec_wall_s"] = t3 - t2
    _last_info["nc"] = nc
    _last_info["in_maps"] = in_maps

    full_q = np.concatenate(outqs, axis=0)         # [8*nsh, 256] int8
    full_s = np.concatenate(outss, axis=0)         # [8*nsh, 1] f32: 127/rowmax
    out_nodes = full_q[prep["gslot"]].astype(np.float32) / full_s[prep["gslot"]]
    total_real = out_nodes[:, :P] + prep["c1"][None, :]
    total_imag = out_nodes[:, P:] + prep["c2"][None, :]
    return total_real, total_imag
